# revision 9
# baseline (speedup 1.0000x reference)
"""Trainium2 Bass kernel for the Dial2vec contrastive loss (nn_Dial2vec).

Math: the dense reference computes, per sequence,
    q = h * a[:,None]; r = h * b[:,None]               (a/b = role-0/1 masks)
    w = q @ r^T; fw = w * band                         (band from turn ids)
    q_cross = fw^T @ q; r_cross = fw @ r
then masked means of q / q_cross / r / r_cross, cosine similarities, and a
label-weighted log-softmax loss.

Because band[i,j] depends only on (turn_i, turn_j) and a*b == 0, everything
collapses to per-turn segment sums over the 16 turns:
    Q_T[t] = sum_{turn_l = t} a_l h_l;  R_T[t] likewise with b     [16, H]
    g_l    = a_l (Band @ R_T)[turn_l] + b_l (Band @ Q_T)[turn_l]   [L, H]
    gam_l  = h_l . g_l
    qs = sum a_l h_l; qc = sum a_l gam_l h_l; rs/rc likewise with b
and cosine similarity is scale-invariant, so the mask-count denominators
cancel (the 1e-8 norm clamps cannot trigger with this data).

The band smear (Band @ ...) is folded into host-precomputed 0/1 matrices:
with ABX[l, 0:16] = b_l * Band[turn_l, :], ABX[l, 16:32] = a_l * Band[turn_l, :],
    g = ABX @ [Q_T; R_T].

Device work per core (one dialogue = 10 sequences, data-parallel over 8
cores): three thin bf16 matmul stages on the PE plus, per 128-token chunk,
one DVE product (gam integrand) and one ACT copy-with-accumulate (the
row-sum). The host performs index-only preprocessing (one-hot / band-smeared
masks, bf16 casts) and the final O(B*H) cosine/log-softmax reduction over
the 40 gathered fp32 vectors per core.
"""

import os

import numpy as np

B_SEQ = 80
L = 512
H = 768
SAMPLES = 10
T = 16
VIEW_RANGE = 2
TEMP = 0.2
AVG_EPS = 1e-6
COS_EPS = 1e-8

N_CORES = 8
SPC = SAMPLES  # sequences per core = one dialogue
P = 128
CHUNKS = L // P  # 4
N_SPLITS = ((0, 512), (512, 768))  # PSUM-bank-aligned fp32 free-dim splits

_CACHE: dict = {}


def _build_nc(repeat: int = 1):
    """Build + compile the per-core Bass program (identical on all cores).

    repeat > 1 emits the whole program body N times (same tensors) — used
    only for wall-clock benchmarking of the steady-state iteration time.
    """
    from contextlib import ExitStack

    import concourse.bacc as bacc
    import concourse.mybir as mybir
    import concourse.tile as tile

    f32 = mybir.dt.float32
    bf16 = mybir.dt.bfloat16
    copy_fn = mybir.ActivationFunctionType.Copy

    nc = bacc.Bacc(
        "TRN2",
        debug=False,
        enable_asserts=False,
        target_bir_lowering=False,
    )

    # one row-block of 128 tokens per chunk; chunks side by side in free dim
    hid = nc.dram_tensor("hid", [SPC, P, CHUNKS * H], bf16, kind="ExternalInput").ap()
    ab = nc.dram_tensor("ab", [SPC, P, CHUNKS * 2 * T], bf16, kind="ExternalInput").ap()
    msk = nc.dram_tensor("msk", [SPC, P, CHUNKS * 2], bf16, kind="ExternalInput").ap()
    abx = nc.dram_tensor("abx", [SPC, 2 * T, L], bf16, kind="ExternalInput").ap()
    out = nc.dram_tensor("out", [4 * SPC, H], f32, kind="ExternalOutput").ap()

    with tile.TileContext(nc) as tc, ExitStack() as ctx:
        hp = ctx.enter_context(tc.tile_pool(name="hp", bufs=3))
        abp = ctx.enter_context(tc.tile_pool(name="abp", bufs=3))
        mp = ctx.enter_context(tc.tile_pool(name="mp", bufs=3))
        axp = ctx.enter_context(tc.tile_pool(name="axp", bufs=3))
        s1p = ctx.enter_context(tc.tile_pool(name="s1p", bufs=2))
        scp = ctx.enter_context(tc.tile_pool(name="scp", bufs=3))
        sc2p = ctx.enter_context(tc.tile_pool(name="sc2p", bufs=3))
        gmp = ctx.enter_context(tc.tile_pool(name="gmp", bufs=8))
        dcp = ctx.enter_context(tc.tile_pool(name="dcp", bufs=8))
        osp = ctx.enter_context(tc.tile_pool(name="osp", bufs=2))
        pp1 = ctx.enter_context(tc.tile_pool(name="pp1", bufs=1, space="PSUM"))
        ppg = ctx.enter_context(tc.tile_pool(name="ppg", bufs=2, space="PSUM"))
        pp3 = ctx.enter_context(tc.tile_pool(name="pp3", bufs=1, space="PSUM"))

        for rep in range(repeat):
            for s in range(SPC):
                # ---- loads (one DMA per tensor per sequence) -----------
                ht = hp.tile([P, CHUNKS * H], bf16, name=f"h{rep}_{s}", tag="h")
                nc.sync.dma_start(ht[:], hid[s])
                abt = abp.tile([P, CHUNKS * 2 * T], bf16, name=f"a{rep}_{s}", tag="a")
                nc.sync.dma_start(abt[:], ab[s])
                mt = mp.tile([P, CHUNKS * 2], bf16, name=f"m{rep}_{s}", tag="m")
                nc.sync.dma_start(mt[:], msk[s])
                axt = axp.tile([2 * T, L], bf16, name=f"ax{rep}_{s}", tag="ax")
                nc.sync.dma_start(axt[:], abx[s])

                def hc(ci, n0=0, n1=H):
                    return ht[:, ci * H + n0 : ci * H + n1]

                # ---- stage A: [Q_T; R_T] = sum_ci [A|B]_ci^T @ h_ci ----
                p1 = pp1.tile([2 * T, H], f32, name=f"p1_{rep}_{s}", tag="p1")
                for ci in range(CHUNKS):
                    for n0, n1 in N_SPLITS:
                        nc.tensor.matmul(
                            p1[:, n0:n1],
                            abt[:, ci * 2 * T : (ci + 1) * 2 * T],
                            hc(ci, n0, n1),
                            start=(ci == 0),
                            stop=(ci == CHUNKS - 1),
                        )
                sb1 = s1p.tile([2 * T, H], bf16, name=f"sb1_{rep}_{s}", tag="sb1")
                nc.scalar.copy(sb1[:], p1[:])

                # ---- stage C: g = ABX @ [Q_T; R_T]; gam = h . g --------
                dcols = []
                for ci in range(CHUNKS):
                    pg = ppg.tile([P, H], f32, name=f"pg{rep}_{s}_{ci}", tag="pg")
                    for n0, n1 in N_SPLITS:
                        nc.tensor.matmul(
                            pg[:, n0:n1],
                            axt[:, ci * P : (ci + 1) * P],
                            sb1[:, n0:n1],
                            start=True,
                            stop=True,
                        )
                    scr = scp.tile([P, H], bf16, name=f"sc{rep}_{s}_{ci}", tag="sc")
                    gam = gmp.tile([P, 1], f32, name=f"g{rep}_{s}_{ci}", tag="g")
                    # gam = sum_h pg * h: DVE does the product, ACT the row-sum
                    nc.vector.tensor_mul(scr[:], pg[:], hc(ci))
                    scr2 = sc2p.tile([P, H], bf16, name=f"s2{rep}_{s}_{ci}", tag="s2")
                    nc.scalar.activation(scr2[:], scr[:], copy_fn, accum_out=gam[:])
                    dcol = dcp.tile([P, 4], bf16, name=f"d{rep}_{s}_{ci}", tag="d")
                    nc.vector.tensor_copy(dcol[:, 0:2], mt[:, ci * 2 : ci * 2 + 2])
                    nc.vector.tensor_scalar_mul(
                        dcol[:, 2:4], mt[:, ci * 2 : ci * 2 + 2], gam[:]
                    )
                    dcols.append(dcol)

                # ---- stage D: [qs; rs; qc; rc] = [a,b,a*gam,b*gam]^T @ h
                p3 = pp3.tile([4, H], f32, name=f"p3_{rep}_{s}", tag="p3")
                for ci in range(CHUNKS):
                    for n0, n1 in N_SPLITS:
                        nc.tensor.matmul(
                            p3[:, n0:n1],
                            dcols[ci][:],
                            hc(ci, n0, n1),
                            start=(ci == 0),
                            stop=(ci == CHUNKS - 1),
                        )
                osb = osp.tile([4, H], f32, name=f"o{rep}_{s}", tag="o")
                nc.scalar.copy(osb[:], p3[:])
                nc.sync.dma_start(out[4 * s : 4 * s + 4, :], osb[:])

    nc.compile()
    return nc


def _prep_core_inputs(hidden_states, attention_mask, role_ids, turn_ids):
    """Per-core input maps: one-hot / band-smeared mask prep (index work only)."""
    import ml_dtypes

    bf16 = ml_dtypes.bfloat16

    am = attention_mask.astype(np.float32)
    a = am * (role_ids == 0)
    b = am * (role_ids == 1)
    onehot = (turn_ids[..., None] == np.arange(T, dtype=turn_ids.dtype)).astype(
        np.float32
    )  # [B, L, T]
    A1 = onehot * a[..., None]
    B1 = onehot * b[..., None]
    band = (
        np.abs(np.arange(T)[:, None] - np.arange(T)[None, :]) <= VIEW_RANGE
    ).astype(np.float32)
    A1b = A1 @ band  # a_l * band[turn_l, :]
    B1b = B1 @ band

    def chunked(x):
        # [10, L, F] -> [10, CHUNKS, P, F] -> [10, P, CHUNKS*F]
        f = x.shape[-1]
        return (
            x.reshape(SPC, CHUNKS, P, f)
            .transpose(0, 2, 1, 3)
            .reshape(SPC, P, CHUNKS * f)
        )

    in_maps = []
    for c in range(N_CORES):
        sl = slice(c * SPC, (c + 1) * SPC)
        in_maps.append(
            {
                "hid": np.ascontiguousarray(chunked(hidden_states[sl])).astype(bf16),
                "ab": np.ascontiguousarray(
                    chunked(np.concatenate([A1[sl], B1[sl]], axis=-1))
                ).astype(bf16),
                "msk": np.ascontiguousarray(
                    chunked(np.stack([a[sl], b[sl]], axis=-1))
                ).astype(bf16),
                "abx": np.ascontiguousarray(
                    np.concatenate([B1b[sl], A1b[sl]], axis=-1).transpose(0, 2, 1)
                ).astype(bf16),
            }
        )
    return in_maps, a.sum(-1), b.sum(-1)


def _finalize(outs, labels, na, nb):
    """Host-side O(B*H) reduction: cosine, log-softmax, label-weighted loss."""
    vecs = np.concatenate(outs, axis=0).astype(np.float64).reshape(-1, 4, H)
    qs = vecs[:, 0] / (na + AVG_EPS)[:, None]
    rs = vecs[:, 1] / (nb + AVG_EPS)[:, None]
    qc = vecs[:, 2] / (nb + AVG_EPS)[:, None]
    rc = vecs[:, 3] / (na + AVG_EPS)[:, None]

    def cos(x, y):
        nx = np.maximum(np.linalg.norm(x, axis=-1), COS_EPS)
        ny = np.maximum(np.linalg.norm(y, axis=-1), COS_EPS)
        return (x * y).sum(-1) / (nx * ny)

    logit_q = (cos(qs, qc) / TEMP).reshape(-1, SAMPLES)
    logit_r = (cos(rs, rc) / TEMP).reshape(-1, SAMPLES)

    def lsm(x):
        m = x.max(-1, keepdims=True)
        e = np.exp(x - m)
        return x - m - np.log(e.sum(-1, keepdims=True))

    lab = labels.astype(np.float64)
    loss_q = -np.mean(lsm(logit_q) * lab)
    loss_r = -np.mean(lsm(logit_r) * lab)
    return np.float32(loss_r + loss_q)


def kernel(hidden_states, labels, attention_mask, role_ids, turn_ids):
    from concourse.bass_utils import run_bass_kernel_spmd

    if "nc" not in _CACHE:
        _CACHE["nc"] = _build_nc()
    nc = _CACHE["nc"]

    in_maps, na, nb = _prep_core_inputs(
        np.asarray(hidden_states),
        np.asarray(attention_mask),
        np.asarray(role_ids),
        np.asarray(turn_ids),
    )
    trace = bool(os.environ.get("BASS_KERNEL_TRACE"))
    res = run_bass_kernel_spmd(
        nc, in_maps, core_ids=list(range(N_CORES)), trace=trace
    )
    if trace:
        _CACHE["last_results"] = res
        print(
            f"[kernel] exec_time_ns={res.exec_time_ns} "
            f"mean_exec_time_ns={res.mean_exec_time_ns}"
        )
    outs = [res.results[c]["out"] for c in range(N_CORES)]
    return _finalize(outs, np.asarray(labels), na, nb)
